# revision 3
# baseline (speedup 1.0000x reference)
"""Trainium2 Bass kernel for the scanned batched vec-mat recurrence.

Math (per batch b):
    c_0 = param + offset                       (same for every batch)
    y_t = c_t @ M[b, t]        (vec [512] x mat [512, 512])
    c_{t+1} = leaky_relu(y_t, 0.1)
    output  = y_31  (last pre-activation)

Sharding: data-parallel over batch, 4 batches per core across 8 cores.

The formulation mirrors the XLA-Neuron reference NEFF so the fp32
overflow pattern (the reference output is largely +-inf) matches: each
matrix block [128, 128] is the stationary operand (fp32 LOW/HIGH
two-pass weights), the carry is a [128, 1] moving column, and the 4
contraction chunks accumulate into PSUM in ascending order.  The carry
stays in column-chunk form [128 partitions, 4 chunks] across steps, so
no per-step transpose is needed.

Raw-bass (not Tile): this walrus build cannot attach semaphore waits to
the LDW half of a self-loading fp32 matmul, so all waits are emitted as
standalone wait_ge instructions and the matmuls stay wait-free.

SBUF matrix tile layout per (b, t): tile[p, j*512 + k] = M[j*128+p, k]
(natural row-major split into 4 contiguous [128, 512] slabs -> fully
coalesced DMA).  Weight block for (out chunk q, contraction chunk j) is
tile[:, j*512 + q*128 :][:128]:
    y[q*128+p'] += sum_p M[j*128+p, q*128+p'] * c[j*128+p]
"""

import numpy as np

import concourse.bass as bass
import concourse.mybir as mybir
from concourse.bass_utils import run_bass_kernel_spmd

FP32 = mybir.dt.float32

B_TOTAL = 32
N_CORES = 8
B_PER_CORE = B_TOTAL // N_CORES  # 4
T = 32
N = 512
NCH = N // 128  # 4 chunks
TILE_F = NCH * N  # 2048 floats per partition per (b, t) tile
W = 12  # matrix prefetch ring depth (1 MiB per slot)
NGROUP = T * B_PER_CORE  # 128 (t, b) groups


def build_nc() -> bass.Bass:
    nc = bass.Bass()
    mats = nc.dram_tensor(
        "mats", [B_PER_CORE, T, NCH, 128, N], FP32, kind="ExternalInput"
    )
    carry0 = nc.dram_tensor("carry0", [128, NCH], FP32, kind="ExternalInput")
    res = nc.dram_tensor("res", [B_PER_CORE, 128, NCH], FP32, kind="ExternalOutput")

    with (
        nc.sbuf_tensor([128, W * TILE_F], FP32) as mat_sb,
        nc.sbuf_tensor([128, NCH], FP32) as c0_sb,
        nc.sbuf_tensor([128, B_PER_CORE * 2 * NCH], FP32) as carry_sb,
        nc.sbuf_tensor([128, B_PER_CORE * NCH], FP32) as tmp_sb,
        nc.sbuf_tensor([128, B_PER_CORE * NCH], FP32) as stage_sb,
        nc.psum_tensor([128, B_PER_CORE * NCH], FP32) as psum,
        nc.semaphore() as dma_mat,
        nc.semaphore() as dma_init,
        nc.semaphore() as pe_done,
        nc.semaphore() as dve_done,
        nc.semaphore() as dma_out,
        nc.Block() as block,
    ):
        def slot(g):
            return mat_sb[:, (g % W) * TILE_F : (g % W + 1) * TILE_F]

        def carry_ap(b, sl):
            o = (b * 2 + sl) * NCH
            return carry_sb[:, o : o + NCH]

        def psY(b):
            return psum[:, b * NCH : (b + 1) * NCH]

        @block.sync
        def _(sync):
            sync.dma_start(c0_sb[:, :], carry0[:, :]).then_inc(dma_init, 16)
            for g in range(NGROUP):
                t, b = divmod(g, B_PER_CORE)
                if g >= W:
                    sync.wait_ge(pe_done, g - W + 1)
                for j in range(NCH):
                    sync.dma_start(
                        slot(g)[:, j * N : (j + 1) * N], mats[b, t, j]
                    ).then_inc(dma_mat, 16)
            sync.wait_ge(dve_done, NGROUP)
            for b in range(B_PER_CORE):
                sync.dma_start(
                    res[b], stage_sb[:, b * NCH : (b + 1) * NCH]
                ).then_inc(dma_out, 16)
            sync.wait_ge(dma_out, 16 * B_PER_CORE)

        @block.tensor
        def _(tensor):
            tensor.wait_ge(dma_init, 16)
            for g in range(NGROUP):
                t, b = divmod(g, B_PER_CORE)
                tensor.wait_ge(dma_mat, 64 * (g + 1))
                if t > 0:
                    # also covers PSUM bank-b reuse (relu of (t-1, b) read it)
                    tensor.wait_ge(dve_done, 4 * (t - 1) + b + 1)
                rhs = c0_sb if t == 0 else carry_ap(b, t % 2)
                mm = None
                for q in range(NCH):
                    for j in range(NCH):
                        base = j * N + q * 128
                        mm = tensor.matmul(
                            psY(b)[:, q : q + 1],
                            lhsT=slot(g)[:, base : base + 128],
                            rhs=rhs[:, j : j + 1],
                            start=(j == 0),
                            stop=(j == NCH - 1),
                        )
                mm.then_inc(pe_done, 1)

        @block.vector
        def _(vector):
            for g in range(NGROUP):
                t, b = divmod(g, B_PER_CORE)
                vector.wait_ge(pe_done, g + 1)
                if t < T - 1:
                    # leaky_relu(y) = max(y, 0.1*y): bitwise identical to
                    # select(y>=0, y, 0.1*y) incl. nan/inf propagation.
                    tb = tmp_sb[:, b * NCH : (b + 1) * NCH]
                    vector.tensor_scalar_mul(tb, psY(b)[:, :], 0.1)
                    # DVE write->read of tb needs an explicit pipeline drain
                    vector.drain()
                    vector.tensor_tensor(
                        carry_ap(b, (t + 1) % 2),
                        psY(b)[:, :],
                        tb,
                        op=mybir.AluOpType.max,
                    ).then_inc(dve_done, 1)
                else:
                    vector.tensor_copy(
                        stage_sb[:, b * NCH : (b + 1) * NCH], psY(b)[:, :]
                    ).then_inc(dve_done, 1)

    return nc


_NC_CACHE = None


def _get_nc():
    global _NC_CACHE
    if _NC_CACHE is None:
        _NC_CACHE = build_nc()
    return _NC_CACHE


def kernel(inp, param, offset):
    inp = np.ascontiguousarray(np.asarray(inp, dtype=np.float32))
    param = np.asarray(param, dtype=np.float32)
    offset = np.asarray(offset, dtype=np.float32)

    y0 = (param + offset).astype(np.float32)
    carry0 = np.ascontiguousarray(y0.reshape(NCH, 128).T)  # [128, 4]

    nc = _get_nc()
    in_maps = []
    for c in range(N_CORES):
        shard = np.ascontiguousarray(
            inp[c * B_PER_CORE : (c + 1) * B_PER_CORE]
        ).reshape(B_PER_CORE, T, NCH, 128, N)
        in_maps.append({"mats": shard, "carry0": carry0})

    r = run_bass_kernel_spmd(nc, in_maps, core_ids=list(range(N_CORES)))
    outs = []
    for c in range(N_CORES):
        res = r.results[c]["res"]  # [4, 128, 4];  res[b, p, q] = y[q*128+p]
        outs.append(np.transpose(res, (0, 2, 1)).reshape(B_PER_CORE, N))
    return np.concatenate(outs, axis=0)
